# revision 14
# baseline (speedup 1.0000x reference)
"""Bass/Trainium2 kernel for CoOccurrenceSemanticGroundingLoss.

Reference computation (B=128, T=1024, V=512, L=20):
  present[b,t] = any_l(gs[b,l]==t); count=sum_b present; valid=(0<count<B)
  tgt[b,t]   = +1 if present&valid else -1
  loss[b]    = mean_{t,v} (logits[b,t,v] - tgt[b,t])^2
  entropy[b,t] = -sum_v p*log(p)

Device strategy (8 cores, SPMD):
  - Shard T across cores (128 t per core), full B=128 on partitions.
  - Host precomputes tsc = -2*tgt (tiny: derived from 10KB grounding_signal)
    so each core's kernel is a pure streaming reduction; the cross-example
    count/valid mask is folded into tsc on host.
  - Entropy per [128(b),512(v)] tile: ACT Ln(p) then DVE
    scalar_tensor_tensor (-p)*ln(p) with accum -> entropy column.
  - Logits moments (S1=sum_v x, S2=sum_v x^2): DVE bn_stats for _JD
    columns per 8-wide group, ACT Square/Copy+accum for the rest
    (engine load balance). The ACT-stat columns stream through their own
    tile (xa) so DVE and ACT each gate only their own input buffers.
  - loss uses sum_v (x-t)^2 = S2 + tsc*S1 + V  (t^2==1); +T*V and the
    1/(T*V) normalization happen on host over the 8 per-core partials.
  - Final groups taper (4,2,1,1) so the compute tail after the last DMA
    byte is one t-column, not a full 8-wide group.
"""

import numpy as np

_B, _T, _V, _M = 128, 1024, 512, 8
_TS = _T // _M   # 128 t-columns per core
_GF = 8          # full-group width (2 MiB DMA per tensor)
_NGF = 15        # full groups: 120 columns
_TAIL = (4, 2, 1, 1)  # tapered tail groups: 8 columns
_JD = 8          # all stats on DVE (ACT-offload measured slower)

_cache = {}
last_results = None  # BassKernelResults of the most recent run (for profiling)


def _get_nc():
    if "nc" in _cache:
        return _cache["nc"]
    import concourse.bacc as bacc
    import concourse.tile as tile
    from concourse import mybir

    f32 = mybir.dt.float32
    Alu = mybir.AluOpType
    Act = mybir.ActivationFunctionType
    nc = bacc.Bacc()

    X = nc.dram_tensor("x_logits", [_B, _TS, _V], f32, kind="ExternalInput")
    P = nc.dram_tensor("p_prior", [_B, _TS, _V], f32, kind="ExternalInput")
    TSC = nc.dram_tensor("tsc", [_B, _TS], f32, kind="ExternalInput")
    ENT = nc.dram_tensor("ent", [_B, _TS], f32, kind="ExternalOutput")
    LOSSP = nc.dram_tensor("lossp", [_B, 1], f32, kind="ExternalOutput")

    nact = _GF - _JD  # ACT-stat columns per full group

    with tile.TileContext(nc) as tc:
        with (
            tc.tile_pool(name="io", bufs=3) as io,
            tc.tile_pool(name="scr", bufs=4) as scrp,
            tc.tile_pool(name="stage", bufs=1) as stage,
        ):
            ts_sb = stage.tile([_B, _TS], f32, tag="ts_sb")
            nc.gpsimd.dma_start(out=ts_sb, in_=TSC[:, :])
            bn_stage = stage.tile([_B, _TS, 6], f32, tag="bn_stage")
            ent_stage = stage.tile([_B, _TS], f32, tag="ent_stage")

            def do_entropy(pt, lg, goff, t0, gw):
                for j in range(gw):
                    scr = scrp.tile([_B, _V], f32, tag="stt_scr")
                    nc.vector.scalar_tensor_tensor(
                        out=scr,
                        in0=pt[:, (goff + j) * _V : (goff + j + 1) * _V],
                        scalar=-1.0,
                        in1=lg[:, (goff + j) * _V : (goff + j + 1) * _V],
                        op0=Alu.mult,
                        op1=Alu.mult,
                        accum_out=ent_stage[:, t0 + j : t0 + j + 1],
                    )

            # 15 full groups: xt (DVE columns) + xa (ACT columns) split DMAs
            for g in range(_NGF):
                t0 = g * _GF
                xt = io.tile([_B, _JD * _V], f32, tag="xt")
                nc.gpsimd.dma_start(out=xt, in_=X[:, t0 : t0 + _JD, :])
                pt = io.tile([_B, _GF * _V], f32, tag="pt")
                nc.gpsimd.dma_start(out=pt, in_=P[:, t0 : t0 + _GF, :])
                lg = io.tile([_B, _GF * _V], f32, tag="lg")
                # p comes from softmax so p >= ~3e-8 > 0 always; Ln(p) is
                # safe without an epsilon bias (bias=0.0 uses the preamble
                # const tensor -> no cross-engine dependency).
                nc.scalar.activation(out=lg, in_=pt, func=Act.Ln)
                for j in range(_JD):
                    nc.vector.bn_stats(
                        out=bn_stage[:, t0 + j, :],
                        in_=xt[:, j * _V : (j + 1) * _V],
                    )
                do_entropy(pt, lg, 0, t0, _GF)

            # tapered tail groups, stats all on DVE (minimal per-group tail)
            t0 = _NGF * _GF
            for gw in _TAIL:
                xt = io.tile([_B, gw * _V], f32, tag="xt")
                nc.gpsimd.dma_start(out=xt, in_=X[:, t0 : t0 + gw, :])
                pt = io.tile([_B, gw * _V], f32, tag="pt")
                nc.gpsimd.dma_start(out=pt, in_=P[:, t0 : t0 + gw, :])
                lg = io.tile([_B, gw * _V], f32, tag="lg")
                nc.scalar.activation(out=lg, in_=pt, func=Act.Ln)
                for j in range(gw):
                    nc.vector.bn_stats(
                        out=bn_stage[:, t0 + j, :],
                        in_=xt[:, j * _V : (j + 1) * _V],
                    )
                do_entropy(pt, lg, 0, t0, gw)
                t0 += gw

            # ---- loss column H[b,t] = tsc*S1 + S2 ----
            # bn_stats even/odd moments (cnt=256 each):
            #   S1 = 256*(m_e+m_o); S2 = w_e+w_o + 256*(m_e^2+m_o^2)
            #   H = 256*(tsc*(m_e+m_o) + m_e^2 + m_o^2) + w_e + w_o
            H = stage.tile([_B, _TS], f32, tag="fx_h")

            def bn_fixup(bn_sl, ts_sl, h_sl, shp):
                m_e = bn_sl[..., 1]
                w_e = bn_sl[..., 2]
                m_o = bn_sl[..., 4]
                w_o = bn_sl[..., 5]
                A = stage.tile(shp, f32, tag="fx_a")
                nc.vector.tensor_add(A, m_e, m_o)
                Bv = stage.tile(shp, f32, tag="fx_b")
                nc.vector.tensor_mul(Bv, A, ts_sl)
                C = stage.tile(shp, f32, tag="fx_c")
                nc.vector.tensor_mul(C, m_e, m_e)
                D = stage.tile(shp, f32, tag="fx_d")
                nc.vector.tensor_mul(D, m_o, m_o)
                E = stage.tile(shp, f32, tag="fx_e")
                nc.vector.tensor_add(E, C, D)
                F = stage.tile(shp, f32, tag="fx_f")
                nc.vector.tensor_add(F, Bv, E)
                Gv = stage.tile(shp, f32, tag="fx_g")
                nc.vector.tensor_add(Gv, w_e, w_o)
                nc.vector.scalar_tensor_tensor(
                    out=h_sl, in0=F, scalar=256.0, in1=Gv,
                    op0=Alu.mult, op1=Alu.add,
                )

            bn_fixup(bn_stage[:, :, :], ts_sb[:, :], H[:, :], [_B, _TS])

            lossp = stage.tile([_B, 1], f32, tag="lossp")
            nc.vector.reduce_sum(lossp, H, axis=mybir.AxisListType.X)

            nc.sync.dma_start(out=ENT[:, :], in_=ent_stage)
            nc.sync.dma_start(out=LOSSP[:, :], in_=lossp)

    nc.compile()
    _cache["nc"] = nc
    return nc


def kernel(
    visual_features=None,
    text_features=None,
    semantic_prior=None,
    semantic_prior_logits=None,
    grounding_signal=None,
    **_unused,
):
    global last_results
    gs = np.asarray(grounding_signal).reshape(_B, -1).astype(np.int64)
    present = np.zeros((_B, _T), dtype=bool)
    present[np.arange(_B)[:, None], gs] = True
    count = present.sum(axis=0)
    valid = (count > 0) & (count < _B)
    tgt = np.where(present & valid[None, :], np.float32(1.0), np.float32(-1.0))
    tsc_full = (-2.0 * tgt).astype(np.float32)  # [B, T]

    lg = np.ascontiguousarray(np.asarray(semantic_prior_logits), dtype=np.float32)
    pr = np.ascontiguousarray(np.asarray(semantic_prior), dtype=np.float32)

    in_maps = []
    for c in range(_M):
        sl = slice(c * _TS, (c + 1) * _TS)
        in_maps.append(
            {
                "x_logits": np.ascontiguousarray(lg[:, sl, :]),
                "p_prior": np.ascontiguousarray(pr[:, sl, :]),
                "tsc": np.ascontiguousarray(tsc_full[:, sl]),
            }
        )

    from concourse.bass_utils import run_bass_kernel_spmd

    nc = _get_nc()
    last_results = run_bass_kernel_spmd(nc, in_maps, core_ids=list(range(_M)))
    res = last_results.results

    ent = np.concatenate([r["ent"] for r in res], axis=1).astype(np.float32)
    lsum = np.sum(
        np.stack([r["lossp"][:, 0] for r in res]).astype(np.float64), axis=0
    )
    tv = float(_T * _V)
    loss = ((lsum + tv) / tv).astype(np.float32)
    return loss, ent


# revision 15
# speedup vs baseline: 1.0357x; 1.0357x over previous
"""Bass/Trainium2 kernel for CoOccurrenceSemanticGroundingLoss.

Reference computation (B=128, T=1024, V=512, L=20):
  present[b,t] = any_l(gs[b,l]==t); count=sum_b present; valid=(0<count<B)
  tgt[b,t]   = +1 if present&valid else -1
  loss[b]    = mean_{t,v} (logits[b,t,v] - tgt[b,t])^2
  entropy[b,t] = -sum_v p*log(p)

Device strategy (8 cores, SPMD):
  - Shard T across cores (128 t per core), full B=128 on partitions.
  - Host precomputes tsc = -2*tgt (tiny: derived from 10KB grounding_signal)
    so each core's kernel is a pure streaming reduction; the cross-example
    count/valid mask is folded into tsc on host.
  - Entropy per [128(b),512(v)] tile: ACT Ln(p) then DVE
    scalar_tensor_tensor (-p)*ln(p) with accum -> entropy column.
  - Logits moments (S1=sum_v x, S2=sum_v x^2): DVE bn_stats for _JD
    columns per 8-wide group, ACT Square/Copy+accum for the rest
    (engine load balance). The ACT-stat columns stream through their own
    tile (xa) so DVE and ACT each gate only their own input buffers.
  - loss uses sum_v (x-t)^2 = S2 + tsc*S1 + V  (t^2==1); +T*V and the
    1/(T*V) normalization happen on host over the 8 per-core partials.
  - Final groups taper (4,2,1,1) so the compute tail after the last DMA
    byte is one t-column, not a full 8-wide group.
"""

import numpy as np

_B, _T, _V, _M = 128, 1024, 512, 8
_TS = _T // _M   # 128 t-columns per core
import os as _os
_GF = 8          # full-group width (2 MiB DMA per tensor)
_NGF = int(_os.environ.get("K_NGF", "16"))   # full groups
_TAIL = tuple(int(x) for x in _os.environ.get("K_TAIL", "").split(",") if x)
_BUFS = int(_os.environ.get("K_BUFS", "2"))
_JD = 8          # all stats on DVE (ACT-offload measured slower)

_cache = {}
last_results = None  # BassKernelResults of the most recent run (for profiling)


def _get_nc():
    if "nc" in _cache:
        return _cache["nc"]
    import concourse.bacc as bacc
    import concourse.tile as tile
    from concourse import mybir

    f32 = mybir.dt.float32
    Alu = mybir.AluOpType
    Act = mybir.ActivationFunctionType
    nc = bacc.Bacc()

    X = nc.dram_tensor("x_logits", [_B, _TS, _V], f32, kind="ExternalInput")
    P = nc.dram_tensor("p_prior", [_B, _TS, _V], f32, kind="ExternalInput")
    TSC = nc.dram_tensor("tsc", [_B, _TS], f32, kind="ExternalInput")
    ENT = nc.dram_tensor("ent", [_B, _TS], f32, kind="ExternalOutput")
    LOSSP = nc.dram_tensor("lossp", [_B, 1], f32, kind="ExternalOutput")

    nact = _GF - _JD  # ACT-stat columns per full group

    with tile.TileContext(nc) as tc:
        with (
            tc.tile_pool(name="io", bufs=_BUFS) as io,
            tc.tile_pool(name="scr", bufs=4) as scrp,
            tc.tile_pool(name="stage", bufs=1) as stage,
        ):
            ts_sb = stage.tile([_B, _TS], f32, tag="ts_sb")
            nc.gpsimd.dma_start(out=ts_sb, in_=TSC[:, :])
            bn_stage = stage.tile([_B, _TS, 6], f32, tag="bn_stage")
            ent_stage = stage.tile([_B, _TS], f32, tag="ent_stage")

            def do_entropy(pt, lg, goff, t0, gw):
                for j in range(gw):
                    scr = scrp.tile([_B, _V], f32, tag="stt_scr")
                    nc.vector.scalar_tensor_tensor(
                        out=scr,
                        in0=pt[:, (goff + j) * _V : (goff + j + 1) * _V],
                        scalar=-1.0,
                        in1=lg[:, (goff + j) * _V : (goff + j + 1) * _V],
                        op0=Alu.mult,
                        op1=Alu.mult,
                        accum_out=ent_stage[:, t0 + j : t0 + j + 1],
                    )

            # 15 full groups: xt (DVE columns) + xa (ACT columns) split DMAs
            for g in range(_NGF):
                t0 = g * _GF
                xt = io.tile([_B, _JD * _V], f32, tag="xt")
                nc.gpsimd.dma_start(out=xt, in_=X[:, t0 : t0 + _JD, :])
                pt = io.tile([_B, _GF * _V], f32, tag="pt")
                nc.gpsimd.dma_start(out=pt, in_=P[:, t0 : t0 + _GF, :])
                lg = io.tile([_B, _GF * _V], f32, tag="lg")
                # p comes from softmax so p >= ~3e-8 > 0 always; Ln(p) is
                # safe without an epsilon bias (bias=0.0 uses the preamble
                # const tensor -> no cross-engine dependency).
                nc.scalar.activation(out=lg, in_=pt, func=Act.Ln)
                for j in range(_JD):
                    nc.vector.bn_stats(
                        out=bn_stage[:, t0 + j, :],
                        in_=xt[:, j * _V : (j + 1) * _V],
                    )
                do_entropy(pt, lg, 0, t0, _GF)

            # tapered tail groups, stats all on DVE (minimal per-group tail)
            t0 = _NGF * _GF
            for gw in _TAIL:
                xt = io.tile([_B, gw * _V], f32, tag="xt")
                nc.gpsimd.dma_start(out=xt, in_=X[:, t0 : t0 + gw, :])
                pt = io.tile([_B, gw * _V], f32, tag="pt")
                nc.gpsimd.dma_start(out=pt, in_=P[:, t0 : t0 + gw, :])
                lg = io.tile([_B, gw * _V], f32, tag="lg")
                nc.scalar.activation(out=lg, in_=pt, func=Act.Ln)
                for j in range(gw):
                    nc.vector.bn_stats(
                        out=bn_stage[:, t0 + j, :],
                        in_=xt[:, j * _V : (j + 1) * _V],
                    )
                do_entropy(pt, lg, 0, t0, gw)
                t0 += gw

            # ---- loss column H[b,t] = tsc*S1 + S2 ----
            # bn_stats even/odd moments (cnt=256 each):
            #   S1 = 256*(m_e+m_o); S2 = w_e+w_o + 256*(m_e^2+m_o^2)
            #   H = 256*(tsc*(m_e+m_o) + m_e^2 + m_o^2) + w_e + w_o
            H = stage.tile([_B, _TS], f32, tag="fx_h")

            def bn_fixup(bn_sl, ts_sl, h_sl, shp):
                m_e = bn_sl[..., 1]
                w_e = bn_sl[..., 2]
                m_o = bn_sl[..., 4]
                w_o = bn_sl[..., 5]
                A = stage.tile(shp, f32, tag="fx_a")
                nc.vector.tensor_add(A, m_e, m_o)
                Bv = stage.tile(shp, f32, tag="fx_b")
                nc.vector.tensor_mul(Bv, A, ts_sl)
                C = stage.tile(shp, f32, tag="fx_c")
                nc.vector.tensor_mul(C, m_e, m_e)
                D = stage.tile(shp, f32, tag="fx_d")
                nc.vector.tensor_mul(D, m_o, m_o)
                E = stage.tile(shp, f32, tag="fx_e")
                nc.vector.tensor_add(E, C, D)
                F = stage.tile(shp, f32, tag="fx_f")
                nc.vector.tensor_add(F, Bv, E)
                Gv = stage.tile(shp, f32, tag="fx_g")
                nc.vector.tensor_add(Gv, w_e, w_o)
                nc.vector.scalar_tensor_tensor(
                    out=h_sl, in0=F, scalar=256.0, in1=Gv,
                    op0=Alu.mult, op1=Alu.add,
                )

            bn_fixup(bn_stage[:, :, :], ts_sb[:, :], H[:, :], [_B, _TS])

            lossp = stage.tile([_B, 1], f32, tag="lossp")
            nc.vector.reduce_sum(lossp, H, axis=mybir.AxisListType.X)

            nc.sync.dma_start(out=ENT[:, :], in_=ent_stage)
            nc.sync.dma_start(out=LOSSP[:, :], in_=lossp)

    nc.compile()
    _cache["nc"] = nc
    return nc


def kernel(
    visual_features=None,
    text_features=None,
    semantic_prior=None,
    semantic_prior_logits=None,
    grounding_signal=None,
    **_unused,
):
    global last_results
    gs = np.asarray(grounding_signal).reshape(_B, -1).astype(np.int64)
    present = np.zeros((_B, _T), dtype=bool)
    present[np.arange(_B)[:, None], gs] = True
    count = present.sum(axis=0)
    valid = (count > 0) & (count < _B)
    tgt = np.where(present & valid[None, :], np.float32(1.0), np.float32(-1.0))
    tsc_full = (-2.0 * tgt).astype(np.float32)  # [B, T]

    lg = np.ascontiguousarray(np.asarray(semantic_prior_logits), dtype=np.float32)
    pr = np.ascontiguousarray(np.asarray(semantic_prior), dtype=np.float32)

    in_maps = []
    for c in range(_M):
        sl = slice(c * _TS, (c + 1) * _TS)
        in_maps.append(
            {
                "x_logits": np.ascontiguousarray(lg[:, sl, :]),
                "p_prior": np.ascontiguousarray(pr[:, sl, :]),
                "tsc": np.ascontiguousarray(tsc_full[:, sl]),
            }
        )

    from concourse.bass_utils import run_bass_kernel_spmd

    nc = _get_nc()
    last_results = run_bass_kernel_spmd(nc, in_maps, core_ids=list(range(_M)))
    res = last_results.results

    ent = np.concatenate([r["ent"] for r in res], axis=1).astype(np.float32)
    lsum = np.sum(
        np.stack([r["lossp"][:, 0] for r in res]).astype(np.float64), axis=0
    )
    tv = float(_T * _V)
    loss = ((lsum + tv) / tv).astype(np.float32)
    return loss, ent


# revision 20
# speedup vs baseline: 1.2454x; 1.2024x over previous
"""Bass/Trainium2 kernel for CoOccurrenceSemanticGroundingLoss (8 cores).

Reference computation (B=128, T=1024, V=512, L=20):
  present[b,t] = any_l(gs[b,l]==t); count=sum_b present; valid=(0<count<B)
  tgt[b,t]   = +1 if present&valid else -1
  loss[b]    = mean_{t,v} (logits[b,t,v] - tgt[b,t])^2       -> [B]
  entropy[b,t] = -sum_v p*log(p)                             -> [B,T]

Strategy:
  - visual_features/text_features are unused by the reference math and
    never touch the device.
  - Shard T across the 8 cores (128 t-columns each), full B=128 on the
    SBUF partition axis. Each core streams its 32 MiB slice of
    semantic_prior_logits and semantic_prior once (memory-bound).
  - The grounding-signal -> targets step is 10 KB of index math; it is
    done on host and folded into a per-core tsc = -2*tgt [B, T/8] f32
    input, so no collective is needed for the (0<count<B) validity mask.
  - Per [128(b), 512(v)] tile:
      * logits moments via one DVE bn_stats (even/odd count/mean/m2
        gives S1=sum_v x and S2=sum_v x^2 in a single pass);
      * entropy via ACT Ln(p) then one DVE scalar_tensor_tensor
        (p*-1)*ln(p) with accum_out -> entropy column. p comes from
        softmax so p >= ~3e-8 > 0 and Ln needs no epsilon bias.
  - loss uses sum_v (x-tgt)^2 = S2 + tsc*S1 + V (since tgt^2 == 1);
    the +T*V term and 1/(T*V) normalization happen on host over the 8
    per-core partial sums.
  - DMA in 2 MiB groups (8 t-columns) on the gpsimd SWDGE path (single
    completion semaphore per transfer; HWDGE fan-out measured slower).
    io bufs=3 gives a 3-group prefetch window.

Measured (NTFF trace, core 0 of 8): ~188 us fast mode, HBM-bound: all
16 SDMA engines at ~26.4 GB/s each (~97% of line rate), DVE ~160 us
busy, ACT ~60 us.
"""

import numpy as np

_B, _T, _V, _M = 128, 1024, 512, 8
_TS = _T // _M   # 128 t-columns per core
_G = 8           # t-columns per DMA group (2 MiB per tensor)
_NG = _TS // _G  # 16 groups

_cache = {}
last_results = None  # BassKernelResults of the most recent run (for profiling)


def _get_nc():
    if "nc" in _cache:
        return _cache["nc"]
    import concourse.bacc as bacc
    import concourse.tile as tile
    from concourse import mybir

    f32 = mybir.dt.float32
    Alu = mybir.AluOpType
    Act = mybir.ActivationFunctionType
    nc = bacc.Bacc()

    X = nc.dram_tensor("x_logits", [_B, _TS, _V], f32, kind="ExternalInput")
    P = nc.dram_tensor("p_prior", [_B, _TS, _V], f32, kind="ExternalInput")
    TSC = nc.dram_tensor("tsc", [_B, _TS], f32, kind="ExternalInput")
    ENT = nc.dram_tensor("ent", [_B, _TS], f32, kind="ExternalOutput")
    LOSSP = nc.dram_tensor("lossp", [_B, 1], f32, kind="ExternalOutput")

    with tile.TileContext(nc) as tc:
        with (
            tc.tile_pool(name="io", bufs=3) as io,
            tc.tile_pool(name="scr", bufs=4) as scrp,
            tc.tile_pool(name="stage", bufs=1) as stage,
        ):
            ts_sb = stage.tile([_B, _TS], f32, tag="ts_sb")
            nc.gpsimd.dma_start(out=ts_sb, in_=TSC[:, :])
            bn_stage = stage.tile([_B, _TS, 6], f32, tag="bn_stage")
            ent_stage = stage.tile([_B, _TS], f32, tag="ent_stage")

            for g in range(_NG):
                t0 = g * _G
                xt = io.tile([_B, _G * _V], f32, tag="xt")
                nc.gpsimd.dma_start(out=xt, in_=X[:, t0 : t0 + _G, :])
                pt = io.tile([_B, _G * _V], f32, tag="pt")
                nc.gpsimd.dma_start(out=pt, in_=P[:, t0 : t0 + _G, :])
                lg = io.tile([_B, _G * _V], f32, tag="lg")
                nc.scalar.activation(out=lg, in_=pt, func=Act.Ln)
                for j in range(_G):
                    t = t0 + j
                    nc.vector.bn_stats(
                        out=bn_stage[:, t, :],
                        in_=xt[:, j * _V : (j + 1) * _V],
                    )
                    scr = scrp.tile([_B, _V], f32, tag="stt_scr")
                    nc.vector.scalar_tensor_tensor(
                        out=scr,
                        in0=pt[:, j * _V : (j + 1) * _V],
                        scalar=-1.0,
                        in1=lg[:, j * _V : (j + 1) * _V],
                        op0=Alu.mult,
                        op1=Alu.mult,
                        accum_out=ent_stage[:, t : t + 1],
                    )

            # ---- loss column H[b,t] = tsc*S1 + S2 ----
            # bn_stats gives even/odd element moments per (b,t):
            #   [cnt_e, m_e, cnt_e*var_e, cnt_o, m_o, cnt_o*var_o], cnt=256
            #   S1 = 256*(m_e+m_o); S2 = w_e+w_o + 256*(m_e^2+m_o^2)
            #   H = 256*(tsc*(m_e+m_o) + m_e^2 + m_o^2) + w_e + w_o
            m_e = bn_stage[:, :, 1]
            w_e = bn_stage[:, :, 2]
            m_o = bn_stage[:, :, 4]
            w_o = bn_stage[:, :, 5]
            shp = [_B, _TS]
            A = stage.tile(shp, f32, tag="fx_a")
            nc.vector.tensor_add(A, m_e, m_o)
            Bv = stage.tile(shp, f32, tag="fx_b")
            nc.vector.tensor_mul(Bv, A, ts_sb)
            C = stage.tile(shp, f32, tag="fx_c")
            nc.vector.tensor_mul(C, m_e, m_e)
            D = stage.tile(shp, f32, tag="fx_d")
            nc.vector.tensor_mul(D, m_o, m_o)
            E = stage.tile(shp, f32, tag="fx_e")
            nc.vector.tensor_add(E, C, D)
            F = stage.tile(shp, f32, tag="fx_f")
            nc.vector.tensor_add(F, Bv, E)
            Gv = stage.tile(shp, f32, tag="fx_g")
            nc.vector.tensor_add(Gv, w_e, w_o)
            H = stage.tile(shp, f32, tag="fx_h")
            nc.vector.scalar_tensor_tensor(
                out=H, in0=F, scalar=256.0, in1=Gv,
                op0=Alu.mult, op1=Alu.add,
            )
            lossp = stage.tile([_B, 1], f32, tag="lossp")
            nc.vector.reduce_sum(lossp, H, axis=mybir.AxisListType.X)

            nc.sync.dma_start(out=ENT[:, :], in_=ent_stage)
            nc.sync.dma_start(out=LOSSP[:, :], in_=lossp)

    nc.compile()
    _cache["nc"] = nc
    return nc


def kernel(
    visual_features=None,
    text_features=None,
    semantic_prior=None,
    semantic_prior_logits=None,
    grounding_signal=None,
    **_unused,
):
    global last_results
    gs = np.asarray(grounding_signal).reshape(_B, -1).astype(np.int64)
    present = np.zeros((_B, _T), dtype=bool)
    present[np.arange(_B)[:, None], gs] = True
    count = present.sum(axis=0)
    valid = (count > 0) & (count < _B)
    tgt = np.where(present & valid[None, :], np.float32(1.0), np.float32(-1.0))
    tsc_full = (-2.0 * tgt).astype(np.float32)  # [B, T]

    lg = np.ascontiguousarray(np.asarray(semantic_prior_logits), dtype=np.float32)
    pr = np.ascontiguousarray(np.asarray(semantic_prior), dtype=np.float32)

    in_maps = []
    for c in range(_M):
        sl = slice(c * _TS, (c + 1) * _TS)
        in_maps.append(
            {
                "x_logits": np.ascontiguousarray(lg[:, sl, :]),
                "p_prior": np.ascontiguousarray(pr[:, sl, :]),
                "tsc": np.ascontiguousarray(tsc_full[:, sl]),
            }
        )

    from concourse.bass_utils import run_bass_kernel_spmd

    nc = _get_nc()
    last_results = run_bass_kernel_spmd(nc, in_maps, core_ids=list(range(_M)))
    res = last_results.results

    ent = np.concatenate([r["ent"] for r in res], axis=1).astype(np.float32)
    lsum = np.sum(
        np.stack([r["lossp"][:, 0] for r in res]).astype(np.float64), axis=0
    )
    tv = float(_T * _V)
    loss = ((lsum + tv) / tv).astype(np.float32)
    return loss, ent
